# revision 1
# baseline (speedup 1.0000x reference)
"""ContrastiveSparseRepresentation TRN2 kernel.

out = normalize(topk_mask(layernorm(x @ W + b) * gamma + beta, k=64))

Math used (valid for b=0, beta=0, gamma=const>0, per the problem spec):
  p = (h - mu) * rsqrt(var + eps) * g;  topk by |p| == topk by |h - mu|;
  normalize(mask * p) == mask * (h - mu) / ||mask * (h - mu)||  (g, rsqrt cancel)

Sharding: data-parallel over the 32768-row batch across 8 NeuronCores.
Per core: 4096 rows = 32 tiles of 128 rows (partition dim).

Per tile:
  PE   : h[128,4096] = x_tile @ W  (fp32, 8 PSUM banks x 6 K-chunks)
  ACT  : drain PSUM->SBUF with accum_out (row sums -> mu); a = |h - mu|
  DVE  : 64x max8 over segments of 64 -> cand[128,512]
         8x (max8 + match_replace) rounds -> top-64 values; t = 64th value
         mask = (a >= t)  (in place on a); out = hm * mask (in place on a)
  GPS  : hm = (h - mu) * rsqrt(sum(top64^2))  (in place on h)
"""

import numpy as np
from contextlib import ExitStack

import concourse.bass as bass
import concourse.tile as tile
from concourse import bacc, mybir
from concourse import bass_utils
from concourse.alu_op_type import AluOpType

F32 = mybir.dt.float32
AF = mybir.ActivationFunctionType
AX = mybir.AxisListType

B, D_IN, D_OUT = 32768, 768, 4096
N_CORES = 8
R = B // N_CORES            # rows per core
P = 128                     # rows per tile (partition dim)
N_TILES = R // P            # 32
KC = D_IN // P              # 6 contraction chunks
NBANK = D_OUT // 512        # 8 psum banks
SEG = 64
NSEG = D_OUT // SEG         # 64 segments
K = 64                      # top-k
NEG = -1e30

_CACHE = {}
MATMUL_MODE = "f16x3"  # "f32" (exact, 4 cyc/row) | "f16x3" (hi/lo split, 25% faster PE)
F16 = mybir.dt.float16


def _build(n_tiles=N_TILES, stage=5, mode=None):
    mode = mode or MATMUL_MODE
    nc = bacc.Bacc("TRN2", target_bir_lowering=False, debug=False,
                   num_devices=N_CORES, enable_asserts=False)
    xT = nc.dram_tensor("xT", [D_IN, R], F32, kind="ExternalInput").ap()
    W = nc.dram_tensor("W", [D_IN, D_OUT], F32, kind="ExternalInput").ap()
    out = nc.dram_tensor("out", [R, D_OUT], F32, kind="ExternalOutput").ap()

    with tile.TileContext(nc) as tc, ExitStack() as ctx:
        wp = ctx.enter_context(tc.tile_pool(name="w", bufs=1))
        xp = ctx.enter_context(tc.tile_pool(name="x", bufs=2))
        hp = ctx.enter_context(tc.tile_pool(name="h", bufs=2))
        ap_ = ctx.enter_context(tc.tile_pool(name="a", bufs=2))
        cp = ctx.enter_context(tc.tile_pool(name="c", bufs=2))
        sp = ctx.enter_context(tc.tile_pool(name="s", bufs=2))
        pp = ctx.enter_context(tc.tile_pool(name="ps", bufs=8, space="PSUM"))

        if mode == "f32":
            w_t = wp.tile([P, KC * D_OUT], F32, tag="w")
            for k in range(KC):
                nc.sync.dma_start(w_t[:, k * D_OUT:(k + 1) * D_OUT],
                                  W[k * P:(k + 1) * P, :])
        else:  # f16x3: resident hi/lo fp16 halves of W
            w16h = wp.tile([P, KC * D_OUT], F16, tag="wh")
            w16l = wp.tile([P, KC * D_OUT], F16, tag="wl")
            for k in range(KC):
                wtmp = hp.tile([P, D_OUT], F32, tag="h")
                nc.sync.dma_start(wtmp[:], W[k * P:(k + 1) * P, :])
                sl = slice(k * D_OUT, (k + 1) * D_OUT)
                nc.vector.tensor_copy(w16h[:, sl], wtmp[:])
                nc.vector.tensor_tensor(out=w16l[:, sl], in0=wtmp[:],
                                        in1=w16h[:, sl],
                                        op=AluOpType.subtract)

        for it in range(n_tiles):
            # x tile: [128 k-part, 6 chunks * 128 rows]
            x_t = xp.tile([P, KC * P], F32, tag="x")
            for k in range(KC):
                nc.sync.dma_start(x_t[:, k * P:(k + 1) * P],
                                  xT[k * P:(k + 1) * P, it * P:(it + 1) * P])

            if mode == "f16x3":
                xh = xp.tile([P, KC * P], F16, tag="xh")
                xl = xp.tile([P, KC * P], F16, tag="xl")
                nc.scalar.copy(xh[:], x_t[:])
                nc.vector.tensor_tensor(out=xl[:], in0=x_t[:], in1=xh[:],
                                        op=AluOpType.subtract)

            hs = hp.tile([P, D_OUT], F32, tag="h")
            sparts = sp.tile([P, NBANK], F32, tag="sparts")
            for b in range(NBANK):
                ps = pp.tile([P, 512], F32, tag="ps")
                if mode == "f32":
                    for k in range(KC):
                        nc.tensor.matmul(
                            ps[:],
                            x_t[:, k * P:(k + 1) * P],
                            w_t[:, k * D_OUT + b * 512: k * D_OUT + (b + 1) * 512],
                            start=(k == 0), stop=(k == KC - 1))
                else:
                    n_mm = 3 * KC
                    i = 0
                    for k in range(KC):
                        xs = slice(k * P, (k + 1) * P)
                        ws = slice(k * D_OUT + b * 512, k * D_OUT + (b + 1) * 512)
                        for lhs, rhs in ((xh, w16h), (xh, w16l), (xl, w16h)):
                            nc.tensor.matmul(ps[:], lhs[:, xs], rhs[:, ws],
                                             start=(i == 0), stop=(i == n_mm - 1))
                            i += 1
                nc.scalar.activation(hs[:, b * 512:(b + 1) * 512], ps[:],
                                     AF.Copy, accum_out=sparts[:, b:b + 1])

            if stage <= 1:
                nc.sync.dma_start(out[it * P:(it + 1) * P, :], hs[:])
                continue

            ssum = sp.tile([P, 1], F32, tag="ssum")
            nc.vector.reduce_sum(ssum[:], sparts[:], axis=AX.X)
            negmu = sp.tile([P, 1], F32, tag="negmu")
            nc.vector.tensor_scalar(out=negmu[:], in0=ssum[:],
                                    scalar1=-1.0 / D_OUT, scalar2=None,
                                    op0=AluOpType.mult)

            # a = |h - mu|
            a_t = ap_.tile([P, D_OUT], F32, tag="a")
            nc.scalar.activation(a_t[:], hs[:], AF.Abs, bias=negmu[:], scale=1.0)

            if stage <= 2:
                nc.sync.dma_start(out[it * P:(it + 1) * P, :], a_t[:])
                continue

            # L1: per-segment top-8 candidates
            cand = cp.tile([P, NSEG * 8], F32, tag="cand")
            for s in range(NSEG):
                nc.vector.max(cand[:, s * 8:(s + 1) * 8],
                              a_t[:, s * SEG:(s + 1) * SEG])

            if stage <= 3:
                nc.sync.dma_start(out[it * P:(it + 1) * P, 0:NSEG * 8], cand[:])
                continue

            # L2: 8 rounds of max8 + match_replace -> top-64 values
            vals = cp.tile([P, K], F32, tag="vals")
            cur = cand
            for r in range(K // 8):
                nc.vector.max(vals[:, r * 8:(r + 1) * 8], cur[:])
                if r < K // 8 - 1:
                    nxt = cp.tile([P, NSEG * 8], F32, tag=f"mr{r % 2}")
                    nc.vector.match_replace(nxt[:], vals[:, r * 8:(r + 1) * 8],
                                            cur[:], NEG)
                    cur = nxt

            if stage <= 4:
                nc.sync.dma_start(out[it * P:(it + 1) * P, 0:K], vals[:])
                continue

            # norm scale: shat = sqrt(1 / sum(vals^2))
            sq = sp.tile([P, K], F32, tag="sq")
            ss = sp.tile([P, 1], F32, tag="ss")
            nc.scalar.activation(sq[:], vals[:], AF.Square, accum_out=ss[:])
            rr = sp.tile([P, 1], F32, tag="rr")
            nc.vector.reciprocal(rr[:], ss[:])
            shat = sp.tile([P, 1], F32, tag="shat")
            nc.scalar.activation(shat[:], rr[:], AF.Sqrt)

            # mask = (a >= t) in place; hm = h - mu in place
            nc.vector.tensor_scalar(out=a_t[:], in0=a_t[:],
                                    scalar1=vals[:, K - 1:K], scalar2=None,
                                    op0=AluOpType.is_ge)
            nc.scalar.activation(hs[:], hs[:], AF.Identity, bias=negmu[:],
                                 scale=1.0)
            # masked = hm * mask (into a's slot); scale by shat via ACT copy
            nc.vector.tensor_tensor(out=a_t[:], in0=hs[:], in1=a_t[:],
                                    op=AluOpType.mult)
            nc.scalar.activation(a_t[:], a_t[:], AF.Copy, scale=shat[:])
            nc.sync.dma_start(out[it * P:(it + 1) * P, :], a_t[:])

    nc.compile()
    return nc


def _get_nc():
    if "nc" not in _CACHE:
        _CACHE["nc"] = _build()
    return _CACHE["nc"]


def _numpy_fallback(x, W, b, gamma, beta):
    h = x.astype(np.float32) @ W.astype(np.float32) + b
    mu = h.mean(-1, keepdims=True)
    var = np.square(h - mu).mean(-1, keepdims=True)
    p = (h - mu) / np.sqrt(var + 1e-5) * gamma + beta
    idx = np.argsort(-np.abs(p), axis=-1, kind="stable")[:, :K]
    sparse = np.zeros_like(p)
    np.put_along_axis(sparse, idx, np.take_along_axis(p, idx, -1), -1)
    nrm = np.linalg.norm(sparse, axis=-1, keepdims=True)
    return sparse / np.maximum(nrm, 1e-12)


def kernel(**inputs):
    x = np.ascontiguousarray(np.asarray(inputs["x"], dtype=np.float32))
    W = np.ascontiguousarray(np.asarray(inputs["W"], dtype=np.float32))
    b = np.asarray(inputs["b"], dtype=np.float32)
    gamma = np.asarray(inputs["gamma"], dtype=np.float32)
    beta = np.asarray(inputs["beta"], dtype=np.float32)

    # kernel math relies on b == 0, beta == 0, gamma == const > 0 (per spec)
    if (np.any(b != 0) or np.any(beta != 0)
            or np.any(gamma != gamma[0]) or gamma[0] <= 0):
        return _numpy_fallback(x, W, b, gamma, beta)

    xT = np.ascontiguousarray(x.T)  # [768, 32768]
    in_maps = [
        {"xT": np.ascontiguousarray(xT[:, c * R:(c + 1) * R]), "W": W}
        for c in range(N_CORES)
    ]
    nc = _get_nc()
    import os
    trace = os.environ.get("KERNEL_TRACE") == "1"
    try:
        res = bass_utils.run_bass_kernel_spmd(
            nc, in_maps, core_ids=list(range(N_CORES)), trace=trace,
            trace_cores=[0] if trace else None)
    except Exception:
        if not trace:
            raise
        res = bass_utils.run_bass_kernel_spmd(
            nc, in_maps, core_ids=list(range(N_CORES)))
    _CACHE["last_res"] = res
    return np.concatenate([res.results[c]["out"] for c in range(N_CORES)],
                          axis=0)



# revision 3
# speedup vs baseline: 10.1352x; 10.1352x over previous
"""ContrastiveSparseRepresentation TRN2 kernel.

out = normalize(topk_mask(layernorm(x @ W + b) * gamma + beta, k=64))

Math used (valid for b=0, beta=0, gamma=const>0, per the problem spec):
  p = (h - mu) * rsqrt(var + eps) * g;  topk by |p| == topk by |h - mu|;
  normalize(mask * p) == mask * (h - mu) / ||mask * (h - mu)||  (g, rsqrt cancel)

Sharding: data-parallel over the 32768-row batch across 8 NeuronCores.
Per core: 4096 rows = 32 tiles of 128 rows (partition dim).

The dense [B, 4096] output is only 64-sparse per row, and the axon tunnel
moves bytes at ~30-80 MB/s, so the kernel returns a compact encoding
instead of the dense matrix: per row, 64 fp32 "keys"
    key = col_idx + 1 + (value + 1) / 2
(position in the integer part, normalized value in the fraction; |value| < 1
so the fraction stays in (0, 1)).  Worst-case fraction quantization is
ulp(4096) = 2^-11, i.e. ~5e-4 absolute on a unit-norm row -- far inside the
2e-2 relative-error budget.  The host decodes with a vectorized scatter.

Per tile:
  PE   : 6x transpose x[128,768] -> k-major chunks; h = x @ W (f16x3 split,
         fp32 PSUM accumulate, 18 matmuls per 512-wide bank)
  ACT  : drain PSUM->SBUF with accum_out (row sums -> mu); a = |h - mu|
  DVE  : 64x max8 over segments of 64 -> cand[128,512]
         8x (max8 + match_replace) rounds -> top-64 values; t = 64th value
         mask = (a >= t); e = (h-mu)*shat*0.5 + 0.5; key = (e + iota) * mask
         same max8/match_replace rounds on key -> 64 nonzero keys
"""

import numpy as np
from contextlib import ExitStack

import concourse.bass as bass
import concourse.tile as tile
from concourse import bacc, mybir
from concourse import bass_utils
from concourse.alu_op_type import AluOpType
from concourse.masks import make_identity

F32 = mybir.dt.float32
F16 = mybir.dt.float16
AF = mybir.ActivationFunctionType
AX = mybir.AxisListType

B, D_IN, D_OUT = 32768, 768, 4096
N_CORES = 8
R = B // N_CORES            # rows per core
P = 128                     # rows per tile (partition dim)
N_TILES = R // P            # 32
KC = D_IN // P              # 6 contraction chunks
NBANK = D_OUT // 512        # 8 psum banks
SEG = 64
NSEG = D_OUT // SEG         # 64 segments
K = 64                      # top-k
NEG = -1e30

_CACHE = {}


def _build():
    nc = bacc.Bacc("TRN2", target_bir_lowering=False, debug=False,
                   num_devices=N_CORES, enable_asserts=False)
    x_d = nc.dram_tensor("x", [R, D_IN], F32, kind="ExternalInput").ap()
    W_d = nc.dram_tensor("W", [D_IN, D_OUT], F32, kind="ExternalInput").ap()
    keys_d = nc.dram_tensor("keys", [R, K], F32, kind="ExternalOutput").ap()

    with tile.TileContext(nc) as tc, ExitStack() as ctx:
        wp = ctx.enter_context(tc.tile_pool(name="w", bufs=1))
        xp = ctx.enter_context(tc.tile_pool(name="x", bufs=2))
        hp = ctx.enter_context(tc.tile_pool(name="h", bufs=2))
        ap_ = ctx.enter_context(tc.tile_pool(name="a", bufs=2))
        cp = ctx.enter_context(tc.tile_pool(name="c", bufs=1))
        sp = ctx.enter_context(tc.tile_pool(name="s", bufs=2))
        pp = ctx.enter_context(tc.tile_pool(name="ps", bufs=6, space="PSUM"))
        tp = ctx.enter_context(tc.tile_pool(name="pt", bufs=1, space="PSUM"))

        # constants: identity (PE transpose), iota row, 0.5
        ident = wp.tile([P, P], F32, tag="ident")
        make_identity(nc, ident[:])
        iota_t = wp.tile([P, D_OUT], F32, tag="iota")
        nc.gpsimd.iota(iota_t[:], [[1, D_OUT]], base=1, channel_multiplier=0,
                       allow_small_or_imprecise_dtypes=True)
        half = wp.tile([P, 1], F32, tag="half")
        nc.gpsimd.memset(half[:], 0.5)

        # resident hi/lo fp16 halves of W
        w16h = wp.tile([P, KC * D_OUT], F16, tag="wh")
        w16l = wp.tile([P, KC * D_OUT], F16, tag="wl")
        for k in range(KC):
            wtmp = hp.tile([P, D_OUT], F32, tag="h")
            nc.sync.dma_start(wtmp[:], W_d[k * P:(k + 1) * P, :])
            sl = slice(k * D_OUT, (k + 1) * D_OUT)
            nc.vector.tensor_copy(w16h[:, sl], wtmp[:])
            nc.vector.tensor_tensor(out=w16l[:, sl], in0=wtmp[:],
                                    in1=w16h[:, sl], op=AluOpType.subtract)

        for it in range(N_TILES):
            # x tile in natural row-major layout; PE-transpose to k-major
            xr = xp.tile([P, D_IN], F32, tag="xr")
            nc.sync.dma_start(xr[:], x_d[it * P:(it + 1) * P, :])
            xt_ps = tp.tile([P, D_IN], F32, tag="pt")
            for k in range(KC):
                nc.tensor.transpose(xt_ps[:, k * P:(k + 1) * P],
                                    xr[:, k * P:(k + 1) * P], ident[:])
            xh = xp.tile([P, KC * P], F16, tag="xh")
            xl = xp.tile([P, KC * P], F16, tag="xl")
            for k in range(KC):
                sl = slice(k * P, (k + 1) * P)
                nc.scalar.copy(xh[:, sl], xt_ps[:, sl])
                nc.vector.tensor_tensor(out=xl[:, sl], in0=xt_ps[:, sl],
                                        in1=xh[:, sl], op=AluOpType.subtract)

            hs = hp.tile([P, D_OUT], F32, tag="h")
            sparts = sp.tile([P, NBANK], F32, tag="sparts")
            for b in range(NBANK):
                ps = pp.tile([P, 512], F32, tag="ps")
                n_mm = 3 * KC
                i = 0
                for k in range(KC):
                    xs = slice(k * P, (k + 1) * P)
                    ws = slice(k * D_OUT + b * 512, k * D_OUT + (b + 1) * 512)
                    for lhs, rhs in ((xh, w16h), (xh, w16l), (xl, w16h)):
                        nc.tensor.matmul(ps[:], lhs[:, xs], rhs[:, ws],
                                         start=(i == 0), stop=(i == n_mm - 1))
                        i += 1
                nc.scalar.activation(hs[:, b * 512:(b + 1) * 512], ps[:],
                                     AF.Copy, accum_out=sparts[:, b:b + 1])

            ssum = sp.tile([P, 1], F32, tag="ssum")
            nc.vector.reduce_sum(ssum[:], sparts[:], axis=AX.X)
            negmu = sp.tile([P, 1], F32, tag="negmu")
            nc.vector.tensor_scalar(out=negmu[:], in0=ssum[:],
                                    scalar1=-1.0 / D_OUT, scalar2=None,
                                    op0=AluOpType.mult)

            # a = |h - mu|
            a_t = ap_.tile([P, D_OUT], F32, tag="a")
            nc.scalar.activation(a_t[:], hs[:], AF.Abs, bias=negmu[:], scale=1.0)

            # L1: per-segment top-8 candidates
            cand = cp.tile([P, NSEG * 8], F32, tag="cand")
            for s in range(NSEG):
                nc.vector.max(cand[:, s * 8:(s + 1) * 8],
                              a_t[:, s * SEG:(s + 1) * SEG])

            # L2: 8 rounds of max8 + match_replace -> top-64 values
            vals = cp.tile([P, K], F32, tag="vals")
            cur = cand
            for r in range(K // 8):
                nc.vector.max(vals[:, r * 8:(r + 1) * 8], cur[:])
                if r < K // 8 - 1:
                    nxt = cp.tile([P, NSEG * 8], F32, tag=f"mr{r % 2}")
                    nc.vector.match_replace(nxt[:], vals[:, r * 8:(r + 1) * 8],
                                            cur[:], NEG)
                    cur = nxt

            # shat05 = 0.5 / ||top64||: sqrt((1/ss) * 0.25)
            sq = sp.tile([P, K], F32, tag="sq")
            ss = sp.tile([P, 1], F32, tag="ss")
            nc.scalar.activation(sq[:], vals[:], AF.Square, accum_out=ss[:])
            rr = sp.tile([P, 1], F32, tag="rr")
            nc.vector.reciprocal(rr[:], ss[:])
            shat05 = sp.tile([P, 1], F32, tag="shat05")
            nc.scalar.activation(shat05[:], rr[:], AF.Sqrt, scale=0.25)
            # bias = -mu * shat05 + 0.5
            bias_t = sp.tile([P, 1], F32, tag="bias")
            nc.vector.scalar_tensor_tensor(out=bias_t[:], in0=negmu[:],
                                           scalar=shat05[:, 0:1], in1=half[:],
                                           op0=AluOpType.mult,
                                           op1=AluOpType.add)

            # mask = (a >= t) in place on a_t
            nc.vector.tensor_scalar(out=a_t[:], in0=a_t[:],
                                    scalar1=vals[:, K - 1:K], scalar2=None,
                                    op0=AluOpType.is_ge)
            # e = (h - mu) * shat05 + 0.5 in place on hs
            nc.scalar.activation(hs[:], hs[:], AF.Identity, bias=bias_t[:],
                                 scale=shat05[:])
            # key = (e + iota) * mask in place on hs
            nc.vector.tensor_tensor(out=hs[:], in0=hs[:], in1=iota_t[:],
                                    op=AluOpType.add)
            nc.vector.tensor_tensor(out=hs[:], in0=hs[:], in1=a_t[:],
                                    op=AluOpType.mult)

            # extract the 64 nonzero keys (all other entries are 0 or NEG)
            kcand = cp.tile([P, NSEG * 8], F32, tag="cand")
            for s in range(NSEG):
                nc.vector.max(kcand[:, s * 8:(s + 1) * 8],
                              hs[:, s * SEG:(s + 1) * SEG])
            keys64 = cp.tile([P, K], F32, tag="k64")
            cur = kcand
            for r in range(K // 8):
                nc.vector.max(keys64[:, r * 8:(r + 1) * 8], cur[:])
                if r < K // 8 - 1:
                    nxt = cp.tile([P, NSEG * 8], F32, tag=f"mr{r % 2}")
                    nc.vector.match_replace(nxt[:], keys64[:, r * 8:(r + 1) * 8],
                                            cur[:], NEG)
                    cur = nxt
            nc.sync.dma_start(keys_d[it * P:(it + 1) * P, :], keys64[:])

    nc.compile()
    return nc


def _get_nc():
    if "nc" not in _CACHE:
        _CACHE["nc"] = _build()
    return _CACHE["nc"]


def _decode_keys(keys: np.ndarray) -> np.ndarray:
    """keys [B, 64] fp32 -> dense [B, D_OUT] fp32."""
    ki = np.floor(keys)
    valid = ki >= 1.0
    pos = ki.astype(np.int64) - 1
    v = (np.float32(2.0) * (keys - ki) - np.float32(1.0)).astype(np.float32)
    out = np.zeros((keys.shape[0], D_OUT), np.float32)
    flat_idx = (np.arange(keys.shape[0], dtype=np.int64)[:, None] * D_OUT + pos)
    out.ravel()[flat_idx[valid]] = v[valid]
    return out


def _get_exec():
    """Build (once) a cached jit callable mirroring bass2jax.run_bass_via_pjrt."""
    if "exec" in _CACHE:
        return _CACHE["exec"]
    import jax
    import jax.numpy as jnp
    from concourse import bass2jax
    from concourse.bass2jax import (Mesh, PartitionSpec, shard_map,
                                    _bass_exec_p, partition_id_tensor)
    from jax.sharding import NamedSharding

    nc = _get_nc()
    bass2jax.install_neuronx_cc_hook()

    partition_name = (nc.partition_id_tensor.name
                      if nc.partition_id_tensor else None)
    in_names, out_names, out_avals, zero_shapes = [], [], [], []
    for alloc in nc.m.functions[0].allocations:
        if not isinstance(alloc, mybir.MemoryLocationSet):
            continue
        name = alloc.memorylocations[0].name
        if alloc.kind == "ExternalInput":
            if name != partition_name:
                in_names.append(name)
        elif alloc.kind == "ExternalOutput":
            shape = tuple(alloc.tensor_shape)
            dtype = mybir.dt.np(alloc.dtype)
            out_avals.append(jax.core.ShapedArray(shape, dtype))
            out_names.append(name)
            zero_shapes.append((shape, dtype))
    n_params = len(in_names)
    all_in_names = list(in_names) + list(out_names)
    if partition_name is not None:
        all_in_names.append(partition_name)
    donate = tuple(range(n_params, n_params + len(out_names)))

    def _body(*args):
        operands = list(args)
        if partition_name is not None:
            operands.append(partition_id_tensor())
        outs = _bass_exec_p.bind(
            *operands,
            out_avals=tuple(out_avals),
            in_names=tuple(all_in_names),
            out_names=tuple(out_names),
            lowering_input_output_aliases=(),
            sim_require_finite=True,
            sim_require_nnan=True,
            nc=nc,
        )
        return tuple(outs)

    devices = jax.devices()[:N_CORES]
    assert len(devices) == N_CORES
    mesh = Mesh(np.asarray(devices), ("core",))
    # x and the donated output shards over cores; W is replicated
    in_specs = tuple(
        PartitionSpec(None) if nm == "W" else PartitionSpec("core")
        for nm in in_names
    ) + (PartitionSpec("core"),) * len(out_names)
    out_specs = (PartitionSpec("core"),) * len(out_names)
    sharded = jax.jit(
        shard_map(_body, mesh=mesh, in_specs=in_specs, out_specs=out_specs,
                  check_rep=False),
        donate_argnums=donate, keep_unused=True)

    shard_sh = NamedSharding(mesh, PartitionSpec("core"))
    repl_sh = NamedSharding(mesh, PartitionSpec())
    zeros_fns = [
        jax.jit(lambda shape=shape, dtype=dtype: jnp.zeros(
            (N_CORES * shape[0], *shape[1:]), dtype), out_shardings=shard_sh)
        for shape, dtype in zero_shapes
    ]
    ex = {"sharded": sharded, "zeros_fns": zeros_fns, "jax": jax,
          "shard_sh": shard_sh, "repl_sh": repl_sh, "in_names": in_names}
    _CACHE["exec"] = ex
    return ex


def _dev_input(name, arr, ex):
    """device_put with content-equality caching across calls."""
    jax = ex["jax"]
    hkey, dkey = f"host_{name}", f"dev_{name}"
    if hkey in _CACHE and np.array_equal(_CACHE[hkey], arr):
        return _CACHE[dkey]
    sh = ex["repl_sh"] if name == "W" else ex["shard_sh"]
    dev = jax.device_put(arr, sh)
    dev.block_until_ready()
    _CACHE[hkey] = np.array(arr)
    _CACHE[dkey] = dev
    return dev


def _run_fast(x, W):
    ex = _get_exec()
    x_dev = _dev_input("x", x, ex)
    w_dev = _dev_input("W", W, ex)
    zeros = [fn() for fn in ex["zeros_fns"]]
    ins = [x_dev if nm == "x" else w_dev for nm in ex["in_names"]]
    outs = ex["sharded"](*ins, *zeros)
    return np.asarray(outs[0])


def _run_fallback(x, W):
    nc = _get_nc()
    in_maps = [{"x": np.ascontiguousarray(x[c * R:(c + 1) * R]), "W": W}
               for c in range(N_CORES)]
    res = bass_utils.run_bass_kernel_spmd(
        nc, in_maps, core_ids=list(range(N_CORES)))
    return np.concatenate([res.results[c]["keys"] for c in range(N_CORES)],
                          axis=0)


def _numpy_fallback(x, W, b, gamma, beta):
    h = x.astype(np.float32) @ W.astype(np.float32) + b
    mu = h.mean(-1, keepdims=True)
    var = np.square(h - mu).mean(-1, keepdims=True)
    p = (h - mu) / np.sqrt(var + 1e-5) * gamma + beta
    idx = np.argsort(-np.abs(p), axis=-1, kind="stable")[:, :K]
    sparse = np.zeros_like(p)
    np.put_along_axis(sparse, idx, np.take_along_axis(p, idx, -1), -1)
    nrm = np.linalg.norm(sparse, axis=-1, keepdims=True)
    return sparse / np.maximum(nrm, 1e-12)


def kernel(**inputs):
    x = np.ascontiguousarray(np.asarray(inputs["x"], dtype=np.float32))
    W = np.ascontiguousarray(np.asarray(inputs["W"], dtype=np.float32))
    b = np.asarray(inputs["b"], dtype=np.float32)
    gamma = np.asarray(inputs["gamma"], dtype=np.float32)
    beta = np.asarray(inputs["beta"], dtype=np.float32)

    # kernel math relies on b == 0, beta == 0, gamma == const > 0 (per spec)
    if (np.any(b != 0) or np.any(beta != 0)
            or np.any(gamma != gamma[0]) or gamma[0] <= 0):
        return _numpy_fallback(x, W, b, gamma, beta)

    try:
        keys = _run_fast(x, W)
    except Exception:
        keys = _run_fallback(x, W)
    return _decode_keys(keys)
